# revision 48
# baseline (speedup 1.0000x reference)
"""Self-contained Trainium2 kernel for nn_CausalTransformerShard.

8-layer GPT-J-style causal transformer (T=1024, D=1024, H=16, F=4096),
last-token logits over V=50257.

Strategy: Megatron tensor-parallel across 8 NeuronCores — heads
column-parallel (2 heads/core), wo row-parallel + AllReduce; MLP w_in
column / w_out row + AllReduce; vocab projection column-parallel with a
host-side concat.  Activations live TRANSPOSED (xT: [D, T], D on
partitions).  LayerNorm mean is folded into the weights host-side
(centered weights: (x-m)@W == x@W_c), so only the per-token rstd scale
remains; it is applied POST-matmul on the small Q/K/V/h outputs, which
decouples the projection matmuls from the LN statistics chain.
Per-token column sums (needed for the mean) ride the AllReduce payload
as an extra row.  Causal masking is a DVE multiply with 4 constant mask
tiles.  Matmul compute in bf16 (f32 accumulate), residual in f32,
AllReduce wire in bf16.  Two 512-column chunks pipeline compute against
the collective stream.
"""

import numpy as np
import ml_dtypes

L, D, H, DH, V, F, T, ROT = 8, 1024, 16, 64, 50257, 4096, 1024, 64
NCORES = 8
HPC = H // NCORES          # heads per core (2)
HD = HPC * DH              # head dims per core (128)
FC = F // NCORES           # ffn dims per core (512)
VP = -(-V // NCORES)       # padded vocab per core (6283)
VPAD = VP * NCORES         # 50264
NKT = D // 128             # K tiles over D (8)
NTT = T // 128             # tiles over T (8)
NFT = FC // 128            # F tiles per core (4)
QCH = 512                  # q-chunk width (PSUM bank)
NQ = T // QCH              # q chunks (2)
EPS = 1e-5
BF = ml_dtypes.bfloat16

_CACHE = {}


def _kmajor(w, p=128):
    """[K, M] -> [p, (K//p)*M]: K-tile kt lives at cols [kt*M:(kt+1)*M]."""
    k, m = w.shape
    return np.ascontiguousarray(
        w.reshape(k // p, p, m).transpose(1, 0, 2).reshape(p, (k // p) * m))


def _build():
    import concourse.bass as bass
    import concourse.bacc as bacc
    import concourse.mybir as mybir
    import concourse.tile as tile
    from concourse.masks import make_identity

    F32, BF16, I32 = mybir.dt.float32, mybir.dt.bfloat16, mybir.dt.int32
    AF = mybir.ActivationFunctionType
    ALU = mybir.AluOpType
    GROUPS = [list(range(NCORES))]

    nc = bacc.Bacc("TRN2", target_bir_lowering=False, debug=False,
                   num_devices=NCORES)

    # ---------------- inputs (per-core shards prepared host-side) ----
    ctx_t = nc.dram_tensor("ctx_t", [128, NTT], I32, kind="ExternalInput")
    embed = nc.dram_tensor("embed", [V, D], BF16, kind="ExternalInput")
    wq_d = nc.dram_tensor("wq", [L, 128, NKT * HD], BF16, kind="ExternalInput")
    wk_d = nc.dram_tensor("wk", [L, 128, NKT * HD], BF16, kind="ExternalInput")
    wv_d = nc.dram_tensor("wv", [L, 128, NKT * HD], BF16, kind="ExternalInput")
    wo_d = nc.dram_tensor("wo", [L, 128, D], BF16, kind="ExternalInput")
    win_d = nc.dram_tensor("win", [L, 128, NKT * FC], BF16,
                           kind="ExternalInput")
    wout_d = nc.dram_tensor("wout", [L, 128, NFT * D], BF16,
                            kind="ExternalInput")
    # rowsum vectors for the AR colsum row
    wors_d = nc.dram_tensor("wors", [L, 128, 1], BF16, kind="ExternalInput")
    wouts_d = nc.dram_tensor("wouts", [L, 128, NFT], BF16,
                             kind="ExternalInput")
    qb_d = nc.dram_tensor("qb", [L, 128, 1], F32, kind="ExternalInput")
    kb_d = nc.dram_tensor("kb", [L, 128, 1], F32, kind="ExternalInput")
    aob_d = nc.dram_tensor("aob", [L, 128, NKT], F32, kind="ExternalInput")
    hb_d = nc.dram_tensor("hb", [L, 128, NFT], F32, kind="ExternalInput")
    mob_d = nc.dram_tensor("mob", [L, 128, NKT], F32, kind="ExternalInput")
    absum_d = nc.dram_tensor("absum", [L, 1, 2], F32, kind="ExternalInput")
    cos_d = nc.dram_tensor("cosT", [128, T], BF16, kind="ExternalInput")
    sin_d = nc.dram_tensor("sinT", [128, T], BF16, kind="ExternalInput")
    pmat_d = nc.dram_tensor("pmat", [128, 128], BF16, kind="ExternalInput")
    mask_d = nc.dram_tensor("maskc", [128, 4 * QCH], BF16,
                            kind="ExternalInput")
    wproj_d = nc.dram_tensor("wproj", [128, NKT * VP], BF16,
                             kind="ExternalInput")
    pb_d = nc.dram_tensor("pb", [1, VP], F32, kind="ExternalInput")
    out_d = nc.dram_tensor("out", [1, VP], F32, kind="ExternalOutput")

    with tile.TileContext(nc) as tc:
        with (
            tc.tile_pool(name="const", bufs=1) as cpool,
            tc.tile_pool(name="x", bufs=1) as xpool,
            tc.tile_pool(name="wts", bufs=2) as wpool,
            tc.tile_pool(name="wfat", bufs=2) as wfat,
            tc.tile_pool(name="big1", bufs=1) as big1,
            tc.tile_pool(name="act", bufs=2) as apool,
            tc.tile_pool(name="ework", bufs=2) as epool,
            tc.tile_pool(name="skinny", bufs=2) as skp,
            tc.tile_pool(name="psA", bufs=4, space="PSUM") as psA,
            tc.tile_pool(name="psB", bufs=2, space="PSUM") as psB,
            tc.tile_pool(name="psC", bufs=2, space="PSUM") as psC,
            tc.tile_pool(name="dram", bufs=4, space="DRAM") as dpool,
        ):
            # ---------------- constants --------------------------------
            ident = cpool.tile([128, 128], F32, name="ident")
            make_identity(nc, ident[:])
            ident_b = cpool.tile([128, 128], BF16, name="ident_b")
            nc.vector.tensor_copy(ident_b[:], ident[:])
            ones_b = cpool.tile([128, 1], BF16, name="ones_b")
            nc.vector.memset(ones_b[:], 1.0)
            one_1 = cpool.tile([1, 1], F32, name="one_1")
            nc.vector.memset(one_1[:], 1.0)
            cos_sb = cpool.tile([128, T], BF16, name="cos_sb")
            nc.sync.dma_start(cos_sb[:], cos_d[:])
            sin_sb = cpool.tile([128, T], BF16, name="sin_sb")
            nc.sync.dma_start(sin_sb[:], sin_d[:])
            pmat = cpool.tile([128, 128], BF16, name="pmat")
            nc.sync.dma_start(pmat[:], pmat_d[:])
            mask_sb = cpool.tile([128, 4 * QCH], BF16, name="mask_sb")
            nc.sync.dma_start(mask_sb[:], mask_d[:])
            eps_t = cpool.tile([128, 1], F32, name="eps_t")
            nc.vector.memset(eps_t[:], EPS)

            # residual stream, transposed: block dt holds D rows
            # [dt*128,(dt+1)*128) x all T positions.
            xT = xpool.tile([128, NKT * T], F32, name="xT")
            # bf16 copy of x (NOT normalized; LN handled via centered
            # weights + post-matmul rstd scale).
            xh = xpool.tile([128, NKT * T], BF16, name="xh")
            xsum = cpool.tile([1, T], F32, name="xsum")

            # persistent V staging (64 cols + 1 ones col per tile)
            v_sb = []
            for h in range(HPC):
                vt = big1.tile([128, NTT * 65], BF16, name=f"v{h}")
                nc.gpsimd.memset(vt[:], 1.0)
                v_sb.append(vt)

            def xT_chunk(c0, w):
                return xT[:].rearrange("p (a t) -> p a t", a=NKT)[
                    :, :, c0:c0 + w]

            def xh_chunk(c0, w):
                return xh[:].rearrange("p (a t) -> p a t", a=NKT)[
                    :, :, c0:c0 + w]

            # ---------------- embedding gather + transpose -------------
            for t in range(NTT):
                idx = skp.tile([128, 1], I32, name="idx", tag="idx")
                nc.sync.dma_start(idx[:], ctx_t[:, t:t + 1])
                xrow = epool.tile([128, D], BF16, name="xrow", tag="xrow",
                                  bufs=2)
                nc.gpsimd.indirect_dma_start(
                    out=xrow[:], out_offset=None, in_=embed[:],
                    in_offset=bass.IndirectOffsetOnAxis(ap=idx[:], axis=0))
                for dt in range(NKT):
                    pst = psA.tile([128, 128], BF16, name="pst", tag="mm")
                    nc.tensor.transpose(
                        pst[:], xrow[:, dt * 128:(dt + 1) * 128],
                        ident_b[:])
                    nc.any.tensor_copy(
                        xT[:, dt * T + t * 128: dt * T + (t + 1) * 128],
                        pst[:])
            for c0 in range(0, T, QCH):
                nc.vector.tensor_copy(xh_chunk(c0, QCH), xT_chunk(c0, QCH))
                ps_i = psC.tile([1, QCH], F32, name="ps_i", tag="st")
                for dt in range(NKT):
                    nc.tensor.matmul(ps_i[:1, :], ones_b[:],
                                     xh[:, dt * T + c0: dt * T + c0 + QCH],
                                     start=(dt == 0), stop=(dt == NKT - 1))
                nc.vector.tensor_copy(xsum[:1, c0:c0 + QCH], ps_i[:1, :])

            # ---------------- helpers ----------------------------------
            def stats_pre(c0, w):
                """Emit the square (ACT) early; PE part comes later so the
                tensor engine stream stays dense."""
                sq = apool.tile([128, NKT * QCH], BF16, name="sq", tag="sq",
                                bufs=1)
                nc.scalar.activation(
                    sq[:].rearrange("p (a t) -> p a t", a=NKT)[:, :, :w],
                    xh_chunk(c0, w), AF.Square)
                return sq

            def stats_mm(sq, c0, w, tagp=""):
                """rb [128,w] f32 bcast of rstd, r [1,w] f32"""
                ps_s2 = psC.tile([1, QCH], F32, name="ps_s2", tag="st")
                for dt in range(NKT):
                    nc.tensor.matmul(ps_s2[:1, :w], ones_b[:],
                                     sq[:, dt * QCH: dt * QCH + w],
                                     start=(dt == 0), stop=(dt == NKT - 1))
                m = skp.tile([1, QCH], F32, name="m", tag="stat")
                nc.vector.tensor_scalar_mul(m[:1, :w], xsum[:1, c0:c0 + w],
                                            1.0 / D)
                m2 = skp.tile([1, QCH], F32, name="m2", tag="stat")
                nc.scalar.activation(m2[:1, :w], m[:1, :w], AF.Square)
                var = skp.tile([1, QCH], F32, name="var", tag="stat")
                nc.vector.scalar_tensor_tensor(
                    var[:1, :w], ps_s2[:1, :w], 1.0 / D, m2[:1, :w],
                    op0=ALU.mult, op1=ALU.subtract)
                sd = skp.tile([1, QCH], F32, name="sd", tag="stat")
                nc.scalar.activation(sd[:1, :w], var[:1, :w], AF.Sqrt,
                                     bias=eps_t[:1, :1])
                r = skp.tile([1, QCH], F32, name="rs", tag="rs" + tagp,
                             bufs=(1 if tagp else 2))
                nc.vector.reciprocal_approx_fast(r[:1, :w], sd[:1, :w])
                rb = apool.tile([128, QCH], F32, name="rb", tag="rb" + tagp)
                nc.gpsimd.partition_broadcast(rb[:, :w], r[:1, :w])
                return rb, r

            def ln_stats(c0, w, tagp=""):
                return stats_mm(stats_pre(c0, w), c0, w, tagp)

            def qk_mm(wmat, c0, w):
                """Projection matmul chain only (PE burst density)."""
                ps = psA.tile([128, QCH], F32, name="psqk", tag="mm")
                for kt in range(NKT):
                    nc.tensor.matmul(
                        ps[:, :w], wmat[:, kt * HD:(kt + 1) * HD],
                        xh[:, kt * T + c0: kt * T + c0 + w],
                        start=(kt == 0), stop=(kt == NKT - 1))
                return ps

            def qk_post(dst, ps, bias, rb, rbo, c0, w):
                """dst[:, c0:c0+w] = rope(r * ps + bias)."""
                t0 = epool.tile([128, QCH], BF16, name="qkt", tag="qkt")
                nc.vector.tensor_mul(t0[:, :w], ps[:, :w],
                                     rb[:, rbo:rbo + w])
                nc.scalar.activation(dst[:, c0:c0 + w], t0[:, :w],
                                     AF.Identity, bias=bias[:])
                # rope in place
                psr = psA.tile([128, QCH], F32, name="psr", tag="mm")
                nc.tensor.matmul(psr[:, :w], pmat[:], dst[:, c0:c0 + w],
                                 start=True, stop=True)
                rsin = epool.tile([128, QCH], BF16, name="rsin", tag="rsin")
                nc.any.tensor_mul(rsin[:, :w], psr[:, :w],
                                  sin_sb[:, c0:c0 + w])
                dcos = epool.tile([128, QCH], BF16, name="dcos", tag="dcos")
                nc.any.tensor_mul(dcos[:, :w], dst[:, c0:c0 + w],
                                  cos_sb[:, c0:c0 + w])
                nc.any.tensor_add(dst[:, c0:c0 + w], dcos[:, :w],
                                  rsin[:, :w])

            def qk_proj(dst, wmat, bias, rb, rbo, c0, w):
                qk_post(dst, qk_mm(wmat, c0, w), bias, rb, rbo, c0, w)

            def v_proj(wv, r, c0, tiles):
                """v_sb[h][τ, 65*t : 65*t+64] = r_τ * (xh^T wv_c).
                rcol transposes live in psB so psA chains never wait on
                the LN-statistics result."""
                for t in tiles:
                    psrc = psB.tile([128, 1], F32, name="psrc", tag="pso")
                    nc.tensor.matmul(psrc[:],
                                     r[0:1, t * 128 - c0:
                                       t * 128 - c0 + 128],
                                     one_1[:], start=True, stop=True)
                    rcol = skp.tile([128, 1], F32, name="rcol", tag="rcol")
                    nc.any.tensor_copy(rcol[:], psrc[:])
                    psv = psA.tile([128, HD], F32, name="psv", tag="mm")
                    for kt in range(NKT):
                        nc.tensor.matmul(
                            psv[:], xh[:, kt * T + t * 128: kt * T +
                                       (t + 1) * 128],
                            wv[:, kt * HD:(kt + 1) * HD],
                            start=(kt == 0), stop=(kt == NKT - 1))
                    for h in range(HPC):
                        nc.vector.tensor_scalar_mul(
                            v_sb[h][:, t * 65: t * 65 + 64],
                            psv[:, h * 64:(h + 1) * 64], rcol[:])

            def attention(kT, qT, oT, c0, w):
                """Head-interleaved: the two heads' K=64 score matmuls sit
                on row-groups 0-63 / 64-127 (base_partition-derived
                tile_position) so the PE overlaps them; per-(head,tile) e
                tiles are small rotating buffers, AV accumulates into two
                PSUM banks as exps complete."""
                kts = [kt for kt in range(NTT) if kt * 128 <= c0 + w - 1]
                nk = len(kts)
                psos = [psB.tile([65, QCH], F32, name=f"pso{h}", tag="pso")
                        for h in range(HPC)]
                for i, kt in enumerate(kts):
                    masked = (kt * 128 + 127 > c0) and w > 1
                    for h in range(HPC):
                        hp = h * 64
                        pss = psA.tile([128, QCH], F32, name="pss",
                                       tag="mm")
                        nc.tensor.matmul(
                            pss[:, :w],
                            kT[hp:hp + 64, kt * 128:(kt + 1) * 128],
                            qT[hp:hp + 64, c0:c0 + w],
                            start=True, stop=True)
                        e = epool.tile([128, QCH], BF16, name="e",
                                       tag="e", bufs=6)
                        nc.scalar.activation(e[:, :w], pss[:, :w], AF.Exp,
                                             scale=1.0 / np.sqrt(DH))
                        if masked:
                            m0 = (kt * 128 - c0)  # 0,128,256,384
                            m0 = (m0 // 128) * QCH
                            nc.vector.tensor_mul(e[:, :w], e[:, :w],
                                                 mask_sb[:, m0:m0 + w])
                        nc.tensor.matmul(
                            psos[h][:, :w],
                            v_sb[h][:, kt * 65:(kt + 1) * 65],
                            e[:, :w],
                            start=(i == 0), stop=(i == nk - 1))
                for h in range(HPC):
                    hp = h * 64
                    pso = psos[h]
                    den = skp.tile([1, QCH], F32, name="den", tag="stat")
                    nc.vector.tensor_copy(den[:1, :w], pso[64:65, :w])
                    rec = skp.tile([1, QCH], F32, name="rec", tag="stat")
                    nc.vector.reciprocal_approx_fast(rec[:1, :w],
                                                     den[:1, :w])
                    recb = skp.tile([1, QCH], BF16, name="recb", tag="recb",
                                    bufs=1)
                    nc.vector.tensor_copy(recb[:1, :w], rec[:1, :w])
                    rcb = epool.tile([64, QCH], BF16, name="rcb", tag="rcb")
                    nc.gpsimd.partition_broadcast(rcb[:, :w], recb[:1, :w])
                    nc.vector.tensor_mul(oT[hp:hp + 64, c0:c0 + w],
                                         pso[:64, :w], rcb[:, :w])

            def block_to_bounce(mm_fn, colsum_fn, w, tag, bias, bias_cs):
                """mm_fn per dt into PSUM -> bf16 stage (+bias/NCORES so the
                reduced payload already carries the full bias); one batched
                DMA to a DRAM bounce [D+1, w]; row D = local colsum."""
                abi = dpool.tile([D + 1, w], BF16, name="abi" + tag,
                                 tag="arin", bufs=4)
                stage = apool.tile([128, NKT * QCH], BF16, name="stg",
                                   tag="stg", bufs=1)
                for dt in range(NKT):
                    ps = psA.tile([128, QCH], F32, name="psdl", tag="mm")
                    mm_fn(ps, dt)
                    nc.any.tensor_scalar_add(
                        stage[:, dt * QCH: dt * QCH + w], ps[:, :w],
                        bias[:, dt:dt + 1])
                nc.sync.dma_start(
                    abi[:D, :].rearrange("(a p) t -> p a t", p=128),
                    stage[:].rearrange("p (a t) -> p a t", a=NKT)[:, :, :w])
                psc = psC.tile([1, QCH], F32, name="pscs", tag="st")
                colsum_fn(psc)
                csb = skp.tile([1, QCH], BF16, name="csb", tag="csb")
                nc.vector.tensor_scalar_add(csb[:1, :w], psc[:1, :w],
                                            bias_cs)
                nc.sync.dma_start(abi[D:D + 1, :], csb[:1, :w])
                return abi

            def ar_start(abi, w):
                """Issue the collective + result-fetch DMAs (no consumers).
                Keeping all cc doorbells in data-ready order on the gpsimd
                queue lets the single collective stream run back-to-back."""
                abo = dpool.tile([D + 1, w], BF16, name="abo", tag="arout",
                                 bufs=4, addr_space="Shared")
                nc.gpsimd.collective_compute(
                    "AllReduce", ALU.add, replica_groups=GROUPS,
                    ins=[abi.opt()], outs=[abo.opt()])
                ds_ = apool.tile([128, NKT * QCH], BF16, name="ds",
                                 tag="dsum")
                nc.sync.dma_start(
                    ds_[:].rearrange("p (a t) -> p a t", a=NKT)[:, :, :w],
                    abo[:D, :].rearrange("(a p) t -> p a t", p=128))
                csr = skp.tile([1, QCH], BF16, name="csr", tag="csr")
                nc.sync.dma_start(csr[:1, :w], abo[D:D + 1, :])
                return ds_, csr

            def ar_finish(st, w, add_c0):
                """Bias already rode the payload: two batched adds.  The
                xh (bf16) add runs first so matmuls unblock ASAP; the f32
                xT update follows (off the critical path)."""
                ds_, csr = st
                dsv = ds_[:].rearrange("p (a t) -> p a t", a=NKT)[:, :, :w]
                nc.vector.tensor_add(xh_chunk(add_c0, w),
                                     xT_chunk(add_c0, w), dsv)
                nc.vector.tensor_add(xT_chunk(add_c0, w),
                                     xT_chunk(add_c0, w), dsv)
                nc.vector.tensor_add(xsum[:1, add_c0:add_c0 + w],
                                     xsum[:1, add_c0:add_c0 + w],
                                     csr[:1, :w])

            # ---------------- transformer layers -----------------------
            for l in range(L):
                last = (l == L - 1)
                wq = wpool.tile([128, NKT * HD], BF16, name="wq", tag="wq")
                nc.sync.dma_start(wq[:], wq_d[l])
                wk = wpool.tile([128, NKT * HD], BF16, name="wk", tag="wk")
                nc.sync.dma_start(wk[:], wk_d[l])
                wv = wpool.tile([128, NKT * HD], BF16, name="wv", tag="wv")
                nc.sync.dma_start(wv[:], wv_d[l])
                wo = wpool.tile([128, D], BF16, name="wo", tag="wo")
                nc.sync.dma_start(wo[:], wo_d[l])
                win = wfat.tile([128, NKT * FC], BF16, name="win", tag="win")
                nc.sync.dma_start(win[:], win_d[l])
                wout = wfat.tile([128, NFT * D], BF16, name="wout",
                                 tag="wout")
                nc.sync.dma_start(wout[:], wout_d[l])
                wors = skp.tile([128, 1], BF16, name="wors", tag="wors")
                nc.sync.dma_start(wors[:], wors_d[l])
                wouts = skp.tile([128, NFT], BF16, name="wouts", tag="wouts")
                nc.sync.dma_start(wouts[:], wouts_d[l])
                qb = skp.tile([128, 1], F32, name="qb", tag="qb")
                nc.sync.dma_start(qb[:], qb_d[l])
                kb = skp.tile([128, 1], F32, name="kb", tag="kb")
                nc.sync.dma_start(kb[:], kb_d[l])
                aob = skp.tile([128, NKT], F32, name="aob", tag="aob")
                nc.sync.dma_start(aob[:], aob_d[l])
                hb = skp.tile([128, NFT], F32, name="hb", tag="hb")
                nc.sync.dma_start(hb[:], hb_d[l])
                mob = skp.tile([128, NKT], F32, name="mob", tag="mob")
                nc.sync.dma_start(mob[:], mob_d[l])
                absum = skp.tile([1, 2], F32, name="absum", tag="absum")
                nc.sync.dma_start(absum[:], absum_d[l])

                kT = big1.tile([128, T], BF16, name="kT", tag="kT")
                qT = big1.tile([128, T], BF16, name="qT", tag="qT")
                oT = big1.tile([128, T], BF16, name="oT", tag="oT")

                def wo_block(c0, w):
                    def attn_mm(ps, dt, c0=c0, w=w):
                        nc.tensor.matmul(
                            ps[:, :w], wo[:, dt * 128:(dt + 1) * 128],
                            oT[:, c0:c0 + w], start=True, stop=True)

                    def attn_cs(psc, c0=c0, w=w):
                        nc.tensor.matmul(psc[:1, :w], wors[:],
                                         oT[:, c0:c0 + w],
                                         start=True, stop=True)

                    return block_to_bounce(attn_mm, attn_cs, w, "a",
                                           aob, absum[:1, 0:1])

                # ---- attention: LN1 + K/V over full T always; Q/attn
                # over full T (or just T-1 for the last layer).  Each
                # chunk's AR is issued right after its payload so the
                # collective stream never idles. ----
                a_sts = []
                if not last:
                    for c in range(NQ):
                        c0 = c * QCH
                        rb, r = ln_stats(c0, QCH)
                        psK = qk_mm(wk, c0, QCH)
                        psQ = qk_mm(wq, c0, QCH)
                        qk_post(kT, psK, kb, rb, 0, c0, QCH)
                        qk_post(qT, psQ, qb, rb, 0, c0, QCH)
                        v_proj(wv, r, c0, range(c0 // 128,
                                                (c0 + QCH) // 128))
                        attention(kT, qT, oT, c0, QCH)
                        a_sts.append(ar_start(wo_block(c0, QCH), QCH))
                    qchunks = [(c * QCH, QCH) for c in range(NQ)]
                else:
                    rbl = None
                    for c in range(NQ):
                        c0 = c * QCH
                        rb, r = ln_stats(c0, QCH)
                        psK = qk_mm(wk, c0, QCH)
                        qk_post(kT, psK, kb, rb, 0, c0, QCH)
                        v_proj(wv, r, c0, range(c0 // 128,
                                                (c0 + QCH) // 128))
                        rbl = rb
                    qk_proj(qT, wq, qb, rbl, QCH - 1, T - 1, 1)
                    attention(kT, qT, oT, T - 1, 1)
                    a_sts.append(ar_start(wo_block(T - 1, 1), 1))
                    qchunks = [(T - 1, 1)]

                # ---- MLP blocks (wait attn AR per chunk) ----
                m_sts = []
                for ci, (c0, w) in enumerate(qchunks):
                    ar_finish(a_sts[ci], w, c0)
                    rb2, _ = ln_stats(c0, w, tagp="2")
                    hT = big1.tile([128, NFT * QCH], BF16, name="hT",
                                   tag="hT", bufs=2)
                    for ft in range(NFT):
                        psh = psA.tile([128, QCH], F32, name="psh", tag="mm")
                        for kt in range(NKT):
                            nc.tensor.matmul(
                                psh[:, :w],
                                win[:, kt * FC + ft * 128: kt * FC +
                                    (ft + 1) * 128],
                                xh[:, kt * T + c0: kt * T + c0 + w],
                                start=(kt == 0), stop=(kt == NKT - 1))
                        hpre = epool.tile([128, QCH], BF16, name="hpre",
                                          tag="hpre")
                        nc.vector.tensor_mul(hpre[:, :w], psh[:, :w],
                                             rb2[:, :w])
                        nc.scalar.activation(
                            hT[:, ft * QCH: ft * QCH + w], hpre[:, :w],
                            AF.Gelu_apprx_tanh, bias=hb[:, ft:ft + 1])

                    def mlp_mm(ps, dt, w=w, hT=hT):
                        for ft in range(NFT):
                            nc.tensor.matmul(
                                ps[:, :w],
                                wout[:, ft * D + dt * 128: ft * D +
                                     (dt + 1) * 128],
                                hT[:, ft * QCH: ft * QCH + w],
                                start=(ft == 0), stop=(ft == NFT - 1))

                    def mlp_cs(psc, w=w, hT=hT):
                        for ft in range(NFT):
                            nc.tensor.matmul(psc[:1, :w],
                                             wouts[:, ft:ft + 1],
                                             hT[:, ft * QCH: ft * QCH + w],
                                             start=(ft == 0),
                                             stop=(ft == NFT - 1))

                    m_sts.append(ar_start(
                        block_to_bounce(mlp_mm, mlp_cs, w, "m",
                                        mob, absum[:1, 1:2]), w))
                for ci, (c0, w) in enumerate(qchunks):
                    ar_finish(m_sts[ci], w, c0)

            # ---------------- final LN (last token) + projection --------
            mf = skp.tile([1, 1], F32, name="mf", tag="fst", bufs=10)
            nc.vector.tensor_scalar_mul(mf[:], xsum[:1, T - 1: T], 1.0 / D)
            ps_f2 = psC.tile([1, 2], F32, name="ps_f2", tag="st")
            for dt in range(NKT):
                sqf = skp.tile([128, 1], BF16, name="sqf", tag="fst", bufs=10)
                nc.scalar.activation(sqf[:], xh[:, dt * T + T - 1: dt * T + T],
                                     AF.Square)
                nc.tensor.matmul(ps_f2[:1, 0:1], ones_b[:], sqf[:],
                                 start=(dt == 0), stop=(dt == NKT - 1))
            mf2 = skp.tile([1, 1], F32, name="mf2", tag="fst", bufs=10)
            nc.scalar.activation(mf2[:], mf[:], AF.Square)
            varf = skp.tile([1, 1], F32, name="varf", tag="fst", bufs=10)
            nc.vector.scalar_tensor_tensor(varf[:], ps_f2[:1, 0:1], 1.0 / D,
                                           mf2[:], op0=ALU.mult,
                                           op1=ALU.subtract)
            sdf = skp.tile([1, 1], F32, name="sdf", tag="fst", bufs=10)
            nc.scalar.activation(sdf[:], varf[:], AF.Sqrt,
                                 bias=eps_t[:1, :1])
            rsf = skp.tile([1, 1], F32, name="rsf", tag="fst", bufs=10)
            nc.vector.reciprocal(rsf[:], sdf[:])
            mfb = skp.tile([128, 1], F32, name="mfb", tag="fst", bufs=10)
            nc.gpsimd.partition_broadcast(mfb[:], mf[:])
            rfb = skp.tile([128, 1], F32, name="rfb", tag="fst", bufs=10)
            nc.gpsimd.partition_broadcast(rfb[:], rsf[:])
            xl = cpool.tile([128, NKT], BF16, name="xl")
            for dt in range(NKT):
                tmpf = skp.tile([128, 1], F32, name="tmpf", tag="fst", bufs=10)
                nc.vector.tensor_sub(tmpf[:],
                                     xT[:, dt * T + T - 1: dt * T + T],
                                     mfb[:])
                nc.vector.tensor_mul(xl[:, dt:dt + 1], tmpf[:], rfb[:])

            vchunks = [(i * QCH, min(QCH, VP - i * QCH))
                       for i in range(-(-VP // QCH))]
            for (v0, vw) in vchunks:
                psp = psC.tile([1, QCH], F32, name="psp", tag="st")
                for kt in range(NKT):
                    wpt = epool.tile([128, QCH], BF16, name="wpt", tag="wpt",
                                     bufs=6)
                    nc.sync.dma_start(wpt[:, :vw],
                                      wproj_d[:, kt * VP + v0: kt * VP + v0
                                              + vw])
                    nc.tensor.matmul(psp[:1, :vw], xl[:, kt:kt + 1],
                                     wpt[:, :vw], start=(kt == 0),
                                     stop=(kt == NKT - 1))
                pbc = skp.tile([1, QCH], F32, name="pbc", tag="stat")
                nc.sync.dma_start(pbc[:1, :vw], pb_d[:, v0:v0 + vw])
                lgc = skp.tile([1, QCH], F32, name="lgc", tag="stat")
                nc.vector.tensor_add(lgc[:1, :vw], psp[:1, :vw],
                                     pbc[:1, :vw])
                nc.sync.dma_start(out_d[:, v0:v0 + vw], lgc[:1, :vw])

    nc.finalize()
    return nc


def _prep_inputs(inputs):
    """Full inputs -> list of 8 per-core input maps (host-side shard)."""
    f32 = np.float32
    ctx = np.asarray(inputs["ctx"])
    embed_w = np.asarray(inputs["embed_w"], f32)
    s1 = np.asarray(inputs["ln1_scale"], f32)
    b1 = np.asarray(inputs["ln1_bias"], f32)
    s2 = np.asarray(inputs["ln2_scale"], f32)
    b2 = np.asarray(inputs["ln2_bias"], f32)
    wq = np.asarray(inputs["wq"], f32)
    wk = np.asarray(inputs["wk"], f32)
    wv = np.asarray(inputs["wv"], f32)
    wo = np.asarray(inputs["wo"], f32)
    win = np.asarray(inputs["w_in"], f32)
    bin_ = np.asarray(inputs["b_in"], f32)
    wout = np.asarray(inputs["w_out"], f32)
    bout = np.asarray(inputs["b_out"], f32)
    sf = np.asarray(inputs["lnf_scale"], f32)
    bf_ = np.asarray(inputs["lnf_bias"], f32)
    wproj = np.asarray(inputs["w_proj"], f32)
    bproj = np.asarray(inputs["b_proj"], f32)

    ctx_t = np.ascontiguousarray(ctx.reshape(NTT, 128).T).astype(np.int32)

    pos = np.arange(T, dtype=f32)
    inv_freq = 1.0 / (10000.0 ** (np.arange(0, ROT, 2, dtype=f32) / ROT))
    freqs = pos[:, None] * inv_freq[None, :]          # [T, 32]
    sin = np.repeat(np.sin(freqs), 2, axis=-1).T      # [64, T]
    cos = np.repeat(np.cos(freqs), 2, axis=-1).T
    sinT = np.ascontiguousarray(np.tile(sin, (2, 1))).astype(BF)
    cosT = np.ascontiguousarray(np.tile(cos, (2, 1))).astype(BF)
    P = np.zeros((128, 128), f32)
    for i in range(64):
        P[2 * i, 2 * i + 1] = -1.0
        P[2 * i + 1, 2 * i] = 1.0
    pmat = np.ascontiguousarray(P.T).astype(BF)

    # causal mask tiles: offset o = 0,128,256,384; M[kp, q] = kp+o <= q
    kp = np.arange(128)[:, None]
    qq = np.arange(QCH)[None, :]
    masks = [(kp + o <= qq).astype(f32) for o in (0, 128, 256, 384)]
    maskc = np.ascontiguousarray(np.concatenate(masks, axis=1)).astype(BF)

    wproj_eff = sf[:, None] * wproj
    pb_full = bf_ @ wproj + bproj                      # [V]
    wproj_pad = np.zeros((D, VPAD), f32)
    wproj_pad[:, :V] = wproj_eff
    pb_pad = np.zeros(VPAD, f32)
    pb_pad[:V] = pb_full

    def center(w_eff):
        return w_eff - w_eff.mean(axis=0, keepdims=True)

    maps = []
    for c in range(NCORES):
        hsl = slice(c * HD, (c + 1) * HD)
        fsl = slice(c * FC, (c + 1) * FC)
        vsl = slice(c * VP, (c + 1) * VP)
        m = {
            "ctx_t": ctx_t,
            "embed": embed_w.astype(BF),
            "cosT": cosT,
            "sinT": sinT,
            "pmat": pmat,
            "maskc": maskc,
            "pb": pb_pad[vsl][None, :].astype(f32),
            "wproj": _kmajor(wproj_pad[:, vsl]).astype(BF),
        }
        wq_l, wk_l, wv_l, wo_l = [], [], [], []
        win_l, wout_l = [], []
        wors_l, wouts_l = [], []
        qb_l, kb_l, aob_l, hb_l, mob_l = [], [], [], [], []
        absum_l = []
        for l in range(L):
            wq_eff = center(s1[l][:, None] * wq[l])
            wk_eff = center(s1[l][:, None] * wk[l])
            wv_eff = center(s1[l][:, None] * wv[l])
            win_eff = center(s2[l][:, None] * win[l])
            wq_l.append(_kmajor(wq_eff[:, hsl]))
            wk_l.append(_kmajor(wk_eff[:, hsl]))
            wv_l.append(_kmajor(wv_eff[:, hsl]))
            wo_l.append(wo[l][hsl, :])
            win_l.append(_kmajor(win_eff[:, fsl]))
            wout_l.append(_kmajor(wout[l][fsl, :]))
            wors_l.append(wo[l][hsl, :].sum(axis=1)[:, None])
            wouts_l.append(
                wout[l][fsl, :].sum(axis=1).reshape(NFT, 128).T)
            qb_l.append((b1[l] @ wq[l])[hsl][:, None])
            kb_l.append((b1[l] @ wk[l])[hsl][:, None])
            aob_full = (b1[l] @ wv[l]) @ wo[l]
            aob_l.append(aob_full.reshape(NKT, 128).T / NCORES)
            hb_l.append(((b2[l] @ win[l]) + bin_[l])[fsl].reshape(NFT,
                                                                  128).T)
            mob_l.append(bout[l].reshape(NKT, 128).T / NCORES)
            absum_l.append(np.array([[aob_full.sum(), bout[l].sum()]],
                                    dtype=f32) / NCORES)
        m["wq"] = np.ascontiguousarray(np.stack(wq_l)).astype(BF)
        m["wk"] = np.ascontiguousarray(np.stack(wk_l)).astype(BF)
        m["wv"] = np.ascontiguousarray(np.stack(wv_l)).astype(BF)
        m["wo"] = np.ascontiguousarray(np.stack(wo_l)).astype(BF)
        m["win"] = np.ascontiguousarray(np.stack(win_l)).astype(BF)
        m["wout"] = np.ascontiguousarray(np.stack(wout_l)).astype(BF)
        m["wors"] = np.ascontiguousarray(np.stack(wors_l)).astype(BF)
        m["wouts"] = np.ascontiguousarray(np.stack(wouts_l)).astype(BF)
        m["qb"] = np.ascontiguousarray(np.stack(qb_l)).astype(f32)
        m["kb"] = np.ascontiguousarray(np.stack(kb_l)).astype(f32)
        m["aob"] = np.ascontiguousarray(np.stack(aob_l)).astype(f32)
        m["hb"] = np.ascontiguousarray(np.stack(hb_l)).astype(f32)
        m["mob"] = np.ascontiguousarray(np.stack(mob_l)).astype(f32)
        m["absum"] = np.ascontiguousarray(np.stack(absum_l)).astype(f32)
        maps.append(m)
    return maps


def _get_compiled():
    if "nc" not in _CACHE:
        _CACHE["nc"] = _build()
    return _CACHE["nc"]


def kernel(**inputs):
    from concourse.bass_utils import run_bass_kernel_spmd
    nc = _get_compiled()
    maps = _prep_inputs(inputs)
    res = run_bass_kernel_spmd(nc, maps, core_ids=list(range(NCORES)),
                               trace=False)
    logits = np.concatenate([res.results[c]["out"][0]
                             for c in range(NCORES)])[:V]
    return logits.reshape(1, 1, V).astype(np.float32)


def run_traced(inputs):
    """Like kernel() but with NTFF tracing; returns (logits, results)."""
    from concourse.bass_utils import run_bass_kernel_spmd
    nc = _get_compiled()
    maps = _prep_inputs(inputs)
    res = run_bass_kernel_spmd(nc, maps, core_ids=list(range(NCORES)),
                               trace=True)
    logits = np.concatenate([res.results[c]["out"][0]
                             for c in range(NCORES)])[:V]
    return logits.reshape(1, 1, V).astype(np.float32), res


# revision 50
# speedup vs baseline: 1.0118x; 1.0118x over previous
"""Self-contained Trainium2 kernel for nn_CausalTransformerShard.

8-layer GPT-J-style causal transformer (T=1024, D=1024, H=16, F=4096),
last-token logits over V=50257.

Strategy: Megatron tensor-parallel across 8 NeuronCores — heads
column-parallel (2 heads/core), wo row-parallel + AllReduce; MLP w_in
column / w_out row + AllReduce; vocab projection column-parallel with a
host-side concat.  Activations live TRANSPOSED (xT: [D, T], D on
partitions).  LayerNorm mean is folded into the weights host-side
(centered weights: (x-m)@W == x@W_c), so only the per-token rstd scale
remains; it is applied POST-matmul on the small Q/K/V/h outputs, which
decouples the projection matmuls from the LN statistics chain.
Per-token column sums (needed for the mean) ride the AllReduce payload
as an extra row.  Causal masking is a DVE multiply with 4 constant mask
tiles.  Matmul compute in bf16 (f32 accumulate), residual in f32,
AllReduce wire in bf16.  Two 512-column chunks pipeline compute against
the collective stream.
"""

import numpy as np
import ml_dtypes

L, D, H, DH, V, F, T, ROT = 8, 1024, 16, 64, 50257, 4096, 1024, 64
NCORES = 8
HPC = H // NCORES          # heads per core (2)
HD = HPC * DH              # head dims per core (128)
FC = F // NCORES           # ffn dims per core (512)
VP = -(-V // NCORES)       # padded vocab per core (6283)
VPAD = VP * NCORES         # 50264
NKT = D // 128             # K tiles over D (8)
NTT = T // 128             # tiles over T (8)
NFT = FC // 128            # F tiles per core (4)
QCH = 512                  # q-chunk width (PSUM bank)
NQ = T // QCH              # q chunks (2)
EPS = 1e-5
BF = ml_dtypes.bfloat16

_CACHE = {}


def _kmajor(w, p=128):
    """[K, M] -> [p, (K//p)*M]: K-tile kt lives at cols [kt*M:(kt+1)*M]."""
    k, m = w.shape
    return np.ascontiguousarray(
        w.reshape(k // p, p, m).transpose(1, 0, 2).reshape(p, (k // p) * m))


def _build():
    import concourse.bass as bass
    import concourse.bacc as bacc
    import concourse.mybir as mybir
    import concourse.tile as tile
    from concourse.masks import make_identity

    F32, BF16, I32 = mybir.dt.float32, mybir.dt.bfloat16, mybir.dt.int32
    AF = mybir.ActivationFunctionType
    ALU = mybir.AluOpType
    GROUPS = [list(range(NCORES))]

    nc = bacc.Bacc("TRN2", target_bir_lowering=False, debug=False,
                   num_devices=NCORES)

    # ---------------- inputs (per-core shards prepared host-side) ----
    ctx_t = nc.dram_tensor("ctx_t", [128, NTT], I32, kind="ExternalInput")
    embed = nc.dram_tensor("embed", [V, D], BF16, kind="ExternalInput")
    wq_d = nc.dram_tensor("wq", [L, 128, NKT * HD], BF16, kind="ExternalInput")
    wk_d = nc.dram_tensor("wk", [L, 128, NKT * HD], BF16, kind="ExternalInput")
    wv_d = nc.dram_tensor("wv", [L, 128, NKT * HD], BF16, kind="ExternalInput")
    wo_d = nc.dram_tensor("wo", [L, 128, D], BF16, kind="ExternalInput")
    win_d = nc.dram_tensor("win", [L, 128, NKT * FC], BF16,
                           kind="ExternalInput")
    wout_d = nc.dram_tensor("wout", [L, 128, NFT * D], BF16,
                            kind="ExternalInput")
    # rowsum vectors for the AR colsum row
    wors_d = nc.dram_tensor("wors", [L, 128, 1], BF16, kind="ExternalInput")
    wouts_d = nc.dram_tensor("wouts", [L, 128, NFT], BF16,
                             kind="ExternalInput")
    qb_d = nc.dram_tensor("qb", [L, 128, 1], F32, kind="ExternalInput")
    kb_d = nc.dram_tensor("kb", [L, 128, 1], F32, kind="ExternalInput")
    aob_d = nc.dram_tensor("aob", [L, 128, NKT], F32, kind="ExternalInput")
    hb_d = nc.dram_tensor("hb", [L, 128, NFT], F32, kind="ExternalInput")
    mob_d = nc.dram_tensor("mob", [L, 128, NKT], F32, kind="ExternalInput")
    absum_d = nc.dram_tensor("absum", [L, 1, 2], F32, kind="ExternalInput")
    cos_d = nc.dram_tensor("cosT", [128, T], BF16, kind="ExternalInput")
    sin_d = nc.dram_tensor("sinT", [128, T], BF16, kind="ExternalInput")
    pmat_d = nc.dram_tensor("pmat", [128, 128], BF16, kind="ExternalInput")
    mask_d = nc.dram_tensor("maskc", [128, 4 * QCH], BF16,
                            kind="ExternalInput")
    wproj_d = nc.dram_tensor("wproj", [128, NKT * VP], BF16,
                             kind="ExternalInput")
    pb_d = nc.dram_tensor("pb", [1, VP], F32, kind="ExternalInput")
    out_d = nc.dram_tensor("out", [1, VP], F32, kind="ExternalOutput")

    with tile.TileContext(nc) as tc:
        with (
            tc.tile_pool(name="const", bufs=1) as cpool,
            tc.tile_pool(name="x", bufs=1) as xpool,
            tc.tile_pool(name="wts", bufs=2) as wpool,
            tc.tile_pool(name="wfat", bufs=2) as wfat,
            tc.tile_pool(name="big1", bufs=1) as big1,
            tc.tile_pool(name="act", bufs=2) as apool,
            tc.tile_pool(name="ework", bufs=2) as epool,
            tc.tile_pool(name="skinny", bufs=2) as skp,
            tc.tile_pool(name="psA", bufs=4, space="PSUM") as psA,
            tc.tile_pool(name="psB", bufs=2, space="PSUM") as psB,
            tc.tile_pool(name="psC", bufs=2, space="PSUM") as psC,
            tc.tile_pool(name="dram", bufs=4, space="DRAM") as dpool,
        ):
            # ---------------- constants --------------------------------
            ident = cpool.tile([128, 128], F32, name="ident")
            make_identity(nc, ident[:])
            ident_b = cpool.tile([128, 128], BF16, name="ident_b")
            nc.vector.tensor_copy(ident_b[:], ident[:])
            ones_b = cpool.tile([128, 1], BF16, name="ones_b")
            nc.vector.memset(ones_b[:], 1.0)
            one_1 = cpool.tile([1, 1], F32, name="one_1")
            nc.vector.memset(one_1[:], 1.0)
            cos_sb = cpool.tile([128, T], BF16, name="cos_sb")
            nc.sync.dma_start(cos_sb[:], cos_d[:])
            sin_sb = cpool.tile([128, T], BF16, name="sin_sb")
            nc.sync.dma_start(sin_sb[:], sin_d[:])
            pmat = cpool.tile([128, 128], BF16, name="pmat")
            nc.sync.dma_start(pmat[:], pmat_d[:])
            mask_sb = cpool.tile([128, 4 * QCH], BF16, name="mask_sb")
            nc.sync.dma_start(mask_sb[:], mask_d[:])
            eps_t = cpool.tile([128, 1], F32, name="eps_t")
            nc.vector.memset(eps_t[:], EPS)

            # residual stream, transposed: block dt holds D rows
            # [dt*128,(dt+1)*128) x all T positions.
            xT = xpool.tile([128, NKT * T], F32, name="xT")
            # bf16 copy of x (NOT normalized; LN handled via centered
            # weights + post-matmul rstd scale).
            xh = xpool.tile([128, NKT * T], BF16, name="xh")
            xsum = cpool.tile([1, T], F32, name="xsum")

            # persistent V staging (64 cols + 1 ones col per tile)
            v_sb = []
            for h in range(HPC):
                vt = big1.tile([128, NTT * 65], BF16, name=f"v{h}")
                nc.gpsimd.memset(vt[:], 1.0)
                v_sb.append(vt)

            def xT_chunk(c0, w):
                return xT[:].rearrange("p (a t) -> p a t", a=NKT)[
                    :, :, c0:c0 + w]

            def xh_chunk(c0, w):
                return xh[:].rearrange("p (a t) -> p a t", a=NKT)[
                    :, :, c0:c0 + w]

            # ---------------- embedding gather + transpose -------------
            for t in range(NTT):
                idx = skp.tile([128, 1], I32, name="idx", tag="idx")
                nc.sync.dma_start(idx[:], ctx_t[:, t:t + 1])
                xrow = epool.tile([128, D], BF16, name="xrow", tag="xrow",
                                  bufs=2)
                nc.gpsimd.indirect_dma_start(
                    out=xrow[:], out_offset=None, in_=embed[:],
                    in_offset=bass.IndirectOffsetOnAxis(ap=idx[:], axis=0))
                for dt in range(NKT):
                    pst = psA.tile([128, 128], BF16, name="pst", tag="mm")
                    nc.tensor.transpose(
                        pst[:], xrow[:, dt * 128:(dt + 1) * 128],
                        ident_b[:])
                    nc.any.tensor_copy(
                        xT[:, dt * T + t * 128: dt * T + (t + 1) * 128],
                        pst[:])
            for c0 in range(0, T, QCH):
                nc.vector.tensor_copy(xh_chunk(c0, QCH), xT_chunk(c0, QCH))
                ps_i = psC.tile([1, QCH], F32, name="ps_i", tag="st")
                for dt in range(NKT):
                    nc.tensor.matmul(ps_i[:1, :], ones_b[:],
                                     xh[:, dt * T + c0: dt * T + c0 + QCH],
                                     start=(dt == 0), stop=(dt == NKT - 1))
                nc.vector.tensor_copy(xsum[:1, c0:c0 + QCH], ps_i[:1, :])

            # ---------------- helpers ----------------------------------
            def stats_pre(c0, w):
                """Emit the square (ACT) early; PE part comes later so the
                tensor engine stream stays dense."""
                sq = apool.tile([128, NKT * QCH], BF16, name="sq", tag="sq",
                                bufs=1)
                nc.scalar.activation(
                    sq[:].rearrange("p (a t) -> p a t", a=NKT)[:, :, :w],
                    xh_chunk(c0, w), AF.Square)
                return sq

            def stats_mm(sq, c0, w, tagp=""):
                """rb [128,w] f32 bcast of rstd, r [1,w] f32"""
                ps_s2 = psC.tile([1, QCH], F32, name="ps_s2", tag="st")
                for dt in range(NKT):
                    nc.tensor.matmul(ps_s2[:1, :w], ones_b[:],
                                     sq[:, dt * QCH: dt * QCH + w],
                                     start=(dt == 0), stop=(dt == NKT - 1))
                m = skp.tile([1, QCH], F32, name="m", tag="stat")
                nc.vector.tensor_scalar_mul(m[:1, :w], xsum[:1, c0:c0 + w],
                                            1.0 / D)
                m2 = skp.tile([1, QCH], F32, name="m2", tag="stat")
                nc.scalar.activation(m2[:1, :w], m[:1, :w], AF.Square)
                var = skp.tile([1, QCH], F32, name="var", tag="stat")
                nc.vector.scalar_tensor_tensor(
                    var[:1, :w], ps_s2[:1, :w], 1.0 / D, m2[:1, :w],
                    op0=ALU.mult, op1=ALU.subtract)
                sd = skp.tile([1, QCH], F32, name="sd", tag="stat")
                nc.scalar.activation(sd[:1, :w], var[:1, :w], AF.Sqrt,
                                     bias=eps_t[:1, :1])
                r = skp.tile([1, QCH], F32, name="rs", tag="rs" + tagp,
                             bufs=(1 if tagp else 2))
                nc.vector.reciprocal_approx_fast(r[:1, :w], sd[:1, :w])
                rb = apool.tile([128, QCH], F32, name="rb", tag="rb" + tagp)
                nc.gpsimd.partition_broadcast(rb[:, :w], r[:1, :w])
                return rb, r

            def ln_stats(c0, w, tagp=""):
                return stats_mm(stats_pre(c0, w), c0, w, tagp)

            def qk_mm(wmat, c0, w):
                """Projection matmul chain only (PE burst density)."""
                ps = psA.tile([128, QCH], F32, name="psqk", tag="mm")
                for kt in range(NKT):
                    nc.tensor.matmul(
                        ps[:, :w], wmat[:, kt * HD:(kt + 1) * HD],
                        xh[:, kt * T + c0: kt * T + c0 + w],
                        start=(kt == 0), stop=(kt == NKT - 1))
                return ps

            def qk_post(dst, ps, bias, rb, rbo, c0, w):
                """dst[:, c0:c0+w] = rope(r * ps + bias)."""
                t0 = epool.tile([128, QCH], BF16, name="qkt", tag="qkt")
                nc.vector.tensor_mul(t0[:, :w], ps[:, :w],
                                     rb[:, rbo:rbo + w])
                nc.scalar.activation(dst[:, c0:c0 + w], t0[:, :w],
                                     AF.Identity, bias=bias[:])
                # rope in place
                psr = psA.tile([128, QCH], F32, name="psr", tag="mm")
                nc.tensor.matmul(psr[:, :w], pmat[:], dst[:, c0:c0 + w],
                                 start=True, stop=True)
                rsin = epool.tile([128, QCH], BF16, name="rsin", tag="rsin")
                nc.any.tensor_mul(rsin[:, :w], psr[:, :w],
                                  sin_sb[:, c0:c0 + w])
                dcos = epool.tile([128, QCH], BF16, name="dcos", tag="dcos")
                nc.any.tensor_mul(dcos[:, :w], dst[:, c0:c0 + w],
                                  cos_sb[:, c0:c0 + w])
                nc.any.tensor_add(dst[:, c0:c0 + w], dcos[:, :w],
                                  rsin[:, :w])

            def qk_proj(dst, wmat, bias, rb, rbo, c0, w):
                qk_post(dst, qk_mm(wmat, c0, w), bias, rb, rbo, c0, w)

            def v_proj(wv, r, c0, tiles):
                """v_sb[h][τ, 65*t : 65*t+64] = r_τ * (xh^T wv_c).
                rcol transposes live in psB so psA chains never wait on
                the LN-statistics result."""
                for t in tiles:
                    psrc = psB.tile([128, 1], F32, name="psrc", tag="pso")
                    nc.tensor.matmul(psrc[:],
                                     r[0:1, t * 128 - c0:
                                       t * 128 - c0 + 128],
                                     one_1[:], start=True, stop=True)
                    rcol = skp.tile([128, 1], F32, name="rcol", tag="rcol")
                    nc.any.tensor_copy(rcol[:], psrc[:])
                    psv = psA.tile([128, HD], F32, name="psv", tag="mm")
                    for kt in range(NKT):
                        nc.tensor.matmul(
                            psv[:], xh[:, kt * T + t * 128: kt * T +
                                       (t + 1) * 128],
                            wv[:, kt * HD:(kt + 1) * HD],
                            start=(kt == 0), stop=(kt == NKT - 1))
                    for h in range(HPC):
                        nc.vector.tensor_scalar_mul(
                            v_sb[h][:, t * 65: t * 65 + 64],
                            psv[:, h * 64:(h + 1) * 64], rcol[:])

            def attention(kT, qT, oT, c0, w):
                """Head-interleaved: the two heads' K=64 score matmuls sit
                on row-groups 0-63 / 64-127 (base_partition-derived
                tile_position) so the PE overlaps them; per-(head,tile) e
                tiles are small rotating buffers, AV accumulates into two
                PSUM banks as exps complete."""
                kts = [kt for kt in range(NTT) if kt * 128 <= c0 + w - 1]
                nk = len(kts)
                psos = [psB.tile([65, QCH], F32, name=f"pso{h}", tag="pso")
                        for h in range(HPC)]
                for i, kt in enumerate(kts):
                    masked = (kt * 128 + 127 > c0) and w > 1
                    for h in range(HPC):
                        hp = h * 64
                        pss = psA.tile([128, QCH], F32, name="pss",
                                       tag="mm")
                        nc.tensor.matmul(
                            pss[:, :w],
                            kT[hp:hp + 64, kt * 128:(kt + 1) * 128],
                            qT[hp:hp + 64, c0:c0 + w],
                            start=True, stop=True)
                        e = epool.tile([128, QCH], BF16, name="e",
                                       tag="e", bufs=4)
                        nc.scalar.activation(e[:, :w], pss[:, :w], AF.Exp,
                                             scale=1.0 / np.sqrt(DH))
                        if masked:
                            m0 = (kt * 128 - c0)  # 0,128,256,384
                            m0 = (m0 // 128) * QCH
                            nc.vector.tensor_mul(e[:, :w], e[:, :w],
                                                 mask_sb[:, m0:m0 + w])
                        nc.tensor.matmul(
                            psos[h][:, :w],
                            v_sb[h][:, kt * 65:(kt + 1) * 65],
                            e[:, :w],
                            start=(i == 0), stop=(i == nk - 1))
                for h in range(HPC):
                    hp = h * 64
                    pso = psos[h]
                    den = skp.tile([1, QCH], F32, name="den", tag="stat")
                    nc.vector.tensor_copy(den[:1, :w], pso[64:65, :w])
                    rec = skp.tile([1, QCH], F32, name="rec", tag="stat")
                    nc.vector.reciprocal_approx_fast(rec[:1, :w],
                                                     den[:1, :w])
                    recb = skp.tile([1, QCH], BF16, name="recb", tag="recb",
                                    bufs=1)
                    nc.vector.tensor_copy(recb[:1, :w], rec[:1, :w])
                    rcb = epool.tile([64, QCH], BF16, name="rcb", tag="rcb")
                    nc.gpsimd.partition_broadcast(rcb[:, :w], recb[:1, :w])
                    nc.vector.tensor_mul(oT[hp:hp + 64, c0:c0 + w],
                                         pso[:64, :w], rcb[:, :w])

            def block_to_bounce(mm_fn, colsum_fn, w, tag, bias, bias_cs):
                """mm_fn per dt into PSUM -> bf16 stage (+bias/NCORES so the
                reduced payload already carries the full bias); one batched
                DMA to a DRAM bounce [D+1, w]; row D = local colsum."""
                abi = dpool.tile([D + 1, w], BF16, name="abi" + tag,
                                 tag="arin", bufs=4)
                # colsum row first: its tiny DMA completes while the eight
                # delta chains run, so the collective only waits on the
                # big stage DMA
                psc = psC.tile([1, QCH], F32, name="pscs", tag="st")
                colsum_fn(psc)
                csb = skp.tile([1, QCH], BF16, name="csb", tag="csb")
                nc.vector.tensor_scalar_add(csb[:1, :w], psc[:1, :w],
                                            bias_cs)
                nc.sync.dma_start(abi[D:D + 1, :], csb[:1, :w])
                stage = apool.tile([128, NKT * QCH], BF16, name="stg",
                                   tag="stg", bufs=1)
                for dt in range(NKT):
                    ps = psA.tile([128, QCH], F32, name="psdl", tag="mm")
                    mm_fn(ps, dt)
                    nc.any.tensor_scalar_add(
                        stage[:, dt * QCH: dt * QCH + w], ps[:, :w],
                        bias[:, dt:dt + 1])
                nc.sync.dma_start(
                    abi[:D, :].rearrange("(a p) t -> p a t", p=128),
                    stage[:].rearrange("p (a t) -> p a t", a=NKT)[:, :, :w])
                return abi

            def ar_start(abi, w):
                """Issue the collective + result-fetch DMAs (no consumers).
                Keeping all cc doorbells in data-ready order on the gpsimd
                queue lets the single collective stream run back-to-back."""
                abo = dpool.tile([D + 1, w], BF16, name="abo", tag="arout",
                                 bufs=4, addr_space="Shared")
                nc.gpsimd.collective_compute(
                    "AllReduce", ALU.add, replica_groups=GROUPS,
                    ins=[abi.opt()], outs=[abo.opt()])
                ds_ = apool.tile([128, NKT * QCH], BF16, name="ds",
                                 tag="dsum")
                nc.sync.dma_start(
                    ds_[:].rearrange("p (a t) -> p a t", a=NKT)[:, :, :w],
                    abo[:D, :].rearrange("(a p) t -> p a t", p=128))
                csr = skp.tile([1, QCH], BF16, name="csr", tag="csr")
                nc.sync.dma_start(csr[:1, :w], abo[D:D + 1, :])
                return ds_, csr

            def ar_finish(st, w, add_c0):
                """Bias already rode the payload: two batched adds.  The
                xh (bf16) add runs first so matmuls unblock ASAP; the f32
                xT update follows (off the critical path)."""
                ds_, csr = st
                dsv = ds_[:].rearrange("p (a t) -> p a t", a=NKT)[:, :, :w]
                nc.vector.tensor_add(xh_chunk(add_c0, w),
                                     xT_chunk(add_c0, w), dsv)
                nc.vector.tensor_add(xT_chunk(add_c0, w),
                                     xT_chunk(add_c0, w), dsv)
                nc.vector.tensor_add(xsum[:1, add_c0:add_c0 + w],
                                     xsum[:1, add_c0:add_c0 + w],
                                     csr[:1, :w])

            # ---------------- transformer layers -----------------------
            for l in range(L):
                last = (l == L - 1)
                wq = wpool.tile([128, NKT * HD], BF16, name="wq", tag="wq")
                nc.sync.dma_start(wq[:], wq_d[l])
                wk = wpool.tile([128, NKT * HD], BF16, name="wk", tag="wk")
                nc.sync.dma_start(wk[:], wk_d[l])
                wv = wpool.tile([128, NKT * HD], BF16, name="wv", tag="wv")
                nc.sync.dma_start(wv[:], wv_d[l])
                wo = wpool.tile([128, D], BF16, name="wo", tag="wo")
                nc.sync.dma_start(wo[:], wo_d[l])
                win = wfat.tile([128, NKT * FC], BF16, name="win", tag="win")
                nc.sync.dma_start(win[:], win_d[l])
                wout = wfat.tile([128, NFT * D], BF16, name="wout",
                                 tag="wout")
                nc.sync.dma_start(wout[:], wout_d[l])
                wors = skp.tile([128, 1], BF16, name="wors", tag="wors")
                nc.sync.dma_start(wors[:], wors_d[l])
                wouts = skp.tile([128, NFT], BF16, name="wouts", tag="wouts")
                nc.sync.dma_start(wouts[:], wouts_d[l])
                qb = skp.tile([128, 1], F32, name="qb", tag="qb")
                nc.sync.dma_start(qb[:], qb_d[l])
                kb = skp.tile([128, 1], F32, name="kb", tag="kb")
                nc.sync.dma_start(kb[:], kb_d[l])
                aob = skp.tile([128, NKT], F32, name="aob", tag="aob")
                nc.sync.dma_start(aob[:], aob_d[l])
                hb = skp.tile([128, NFT], F32, name="hb", tag="hb")
                nc.sync.dma_start(hb[:], hb_d[l])
                mob = skp.tile([128, NKT], F32, name="mob", tag="mob")
                nc.sync.dma_start(mob[:], mob_d[l])
                absum = skp.tile([1, 2], F32, name="absum", tag="absum")
                nc.sync.dma_start(absum[:], absum_d[l])

                kT = big1.tile([128, T], BF16, name="kT", tag="kT")
                qT = big1.tile([128, T], BF16, name="qT", tag="qT")
                oT = big1.tile([128, T], BF16, name="oT", tag="oT")

                def wo_block(c0, w):
                    def attn_mm(ps, dt, c0=c0, w=w):
                        nc.tensor.matmul(
                            ps[:, :w], wo[:, dt * 128:(dt + 1) * 128],
                            oT[:, c0:c0 + w], start=True, stop=True)

                    def attn_cs(psc, c0=c0, w=w):
                        nc.tensor.matmul(psc[:1, :w], wors[:],
                                         oT[:, c0:c0 + w],
                                         start=True, stop=True)

                    return block_to_bounce(attn_mm, attn_cs, w, "a",
                                           aob, absum[:1, 0:1])

                # ---- attention: LN1 + K/V over full T always; Q/attn
                # over full T (or just T-1 for the last layer).  Each
                # chunk's AR is issued right after its payload so the
                # collective stream never idles. ----
                a_sts = []
                if not last:
                    for c in range(NQ):
                        c0 = c * QCH
                        rb, r = ln_stats(c0, QCH)
                        psK = qk_mm(wk, c0, QCH)
                        psQ = qk_mm(wq, c0, QCH)
                        qk_post(kT, psK, kb, rb, 0, c0, QCH)
                        qk_post(qT, psQ, qb, rb, 0, c0, QCH)
                        v_proj(wv, r, c0, range(c0 // 128,
                                                (c0 + QCH) // 128))
                        attention(kT, qT, oT, c0, QCH)
                        a_sts.append(ar_start(wo_block(c0, QCH), QCH))
                    qchunks = [(c * QCH, QCH) for c in range(NQ)]
                else:
                    rbl = None
                    for c in range(NQ):
                        c0 = c * QCH
                        rb, r = ln_stats(c0, QCH)
                        psK = qk_mm(wk, c0, QCH)
                        qk_post(kT, psK, kb, rb, 0, c0, QCH)
                        v_proj(wv, r, c0, range(c0 // 128,
                                                (c0 + QCH) // 128))
                        rbl = rb
                    qk_proj(qT, wq, qb, rbl, QCH - 1, T - 1, 1)
                    attention(kT, qT, oT, T - 1, 1)
                    a_sts.append(ar_start(wo_block(T - 1, 1), 1))
                    qchunks = [(T - 1, 1)]

                # ---- MLP blocks (wait attn AR per chunk) ----
                m_sts = []
                for ci, (c0, w) in enumerate(qchunks):
                    ar_finish(a_sts[ci], w, c0)
                    rb2, _ = ln_stats(c0, w, tagp="2")
                    hT = big1.tile([128, NFT * QCH], BF16, name="hT",
                                   tag="hT", bufs=2)
                    for ft in range(NFT):
                        psh = psA.tile([128, QCH], F32, name="psh", tag="mm")
                        for kt in range(NKT):
                            nc.tensor.matmul(
                                psh[:, :w],
                                win[:, kt * FC + ft * 128: kt * FC +
                                    (ft + 1) * 128],
                                xh[:, kt * T + c0: kt * T + c0 + w],
                                start=(kt == 0), stop=(kt == NKT - 1))
                        hpre = epool.tile([128, QCH], BF16, name="hpre",
                                          tag="hpre")
                        nc.vector.tensor_mul(hpre[:, :w], psh[:, :w],
                                             rb2[:, :w])
                        nc.scalar.activation(
                            hT[:, ft * QCH: ft * QCH + w], hpre[:, :w],
                            AF.Gelu_apprx_tanh, bias=hb[:, ft:ft + 1])

                    def mlp_mm(ps, dt, w=w, hT=hT):
                        for ft in range(NFT):
                            nc.tensor.matmul(
                                ps[:, :w],
                                wout[:, ft * D + dt * 128: ft * D +
                                     (dt + 1) * 128],
                                hT[:, ft * QCH: ft * QCH + w],
                                start=(ft == 0), stop=(ft == NFT - 1))

                    def mlp_cs(psc, w=w, hT=hT):
                        for ft in range(NFT):
                            nc.tensor.matmul(psc[:1, :w],
                                             wouts[:, ft:ft + 1],
                                             hT[:, ft * QCH: ft * QCH + w],
                                             start=(ft == 0),
                                             stop=(ft == NFT - 1))

                    m_sts.append(ar_start(
                        block_to_bounce(mlp_mm, mlp_cs, w, "m",
                                        mob, absum[:1, 1:2]), w))
                for ci, (c0, w) in enumerate(qchunks):
                    ar_finish(m_sts[ci], w, c0)

            # ---------------- final LN (last token) + projection --------
            mf = skp.tile([1, 1], F32, name="mf", tag="fst", bufs=10)
            nc.vector.tensor_scalar_mul(mf[:], xsum[:1, T - 1: T], 1.0 / D)
            ps_f2 = psC.tile([1, 2], F32, name="ps_f2", tag="st")
            for dt in range(NKT):
                sqf = skp.tile([128, 1], BF16, name="sqf", tag="fst", bufs=10)
                nc.scalar.activation(sqf[:], xh[:, dt * T + T - 1: dt * T + T],
                                     AF.Square)
                nc.tensor.matmul(ps_f2[:1, 0:1], ones_b[:], sqf[:],
                                 start=(dt == 0), stop=(dt == NKT - 1))
            mf2 = skp.tile([1, 1], F32, name="mf2", tag="fst", bufs=10)
            nc.scalar.activation(mf2[:], mf[:], AF.Square)
            varf = skp.tile([1, 1], F32, name="varf", tag="fst", bufs=10)
            nc.vector.scalar_tensor_tensor(varf[:], ps_f2[:1, 0:1], 1.0 / D,
                                           mf2[:], op0=ALU.mult,
                                           op1=ALU.subtract)
            sdf = skp.tile([1, 1], F32, name="sdf", tag="fst", bufs=10)
            nc.scalar.activation(sdf[:], varf[:], AF.Sqrt,
                                 bias=eps_t[:1, :1])
            rsf = skp.tile([1, 1], F32, name="rsf", tag="fst", bufs=10)
            nc.vector.reciprocal(rsf[:], sdf[:])
            mfb = skp.tile([128, 1], F32, name="mfb", tag="fst", bufs=10)
            nc.gpsimd.partition_broadcast(mfb[:], mf[:])
            rfb = skp.tile([128, 1], F32, name="rfb", tag="fst", bufs=10)
            nc.gpsimd.partition_broadcast(rfb[:], rsf[:])
            xl = cpool.tile([128, NKT], BF16, name="xl")
            for dt in range(NKT):
                tmpf = skp.tile([128, 1], F32, name="tmpf", tag="fst", bufs=10)
                nc.vector.tensor_sub(tmpf[:],
                                     xT[:, dt * T + T - 1: dt * T + T],
                                     mfb[:])
                nc.vector.tensor_mul(xl[:, dt:dt + 1], tmpf[:], rfb[:])

            vchunks = [(i * QCH, min(QCH, VP - i * QCH))
                       for i in range(-(-VP // QCH))]
            for (v0, vw) in vchunks:
                psp = psC.tile([1, QCH], F32, name="psp", tag="st")
                for kt in range(NKT):
                    wpt = epool.tile([128, QCH], BF16, name="wpt", tag="wpt",
                                     bufs=6)
                    nc.sync.dma_start(wpt[:, :vw],
                                      wproj_d[:, kt * VP + v0: kt * VP + v0
                                              + vw])
                    nc.tensor.matmul(psp[:1, :vw], xl[:, kt:kt + 1],
                                     wpt[:, :vw], start=(kt == 0),
                                     stop=(kt == NKT - 1))
                pbc = skp.tile([1, QCH], F32, name="pbc", tag="stat")
                nc.sync.dma_start(pbc[:1, :vw], pb_d[:, v0:v0 + vw])
                lgc = skp.tile([1, QCH], F32, name="lgc", tag="stat")
                nc.vector.tensor_add(lgc[:1, :vw], psp[:1, :vw],
                                     pbc[:1, :vw])
                nc.sync.dma_start(out_d[:, v0:v0 + vw], lgc[:1, :vw])

    nc.finalize()
    return nc


def _prep_inputs(inputs):
    """Full inputs -> list of 8 per-core input maps (host-side shard)."""
    f32 = np.float32
    ctx = np.asarray(inputs["ctx"])
    embed_w = np.asarray(inputs["embed_w"], f32)
    s1 = np.asarray(inputs["ln1_scale"], f32)
    b1 = np.asarray(inputs["ln1_bias"], f32)
    s2 = np.asarray(inputs["ln2_scale"], f32)
    b2 = np.asarray(inputs["ln2_bias"], f32)
    wq = np.asarray(inputs["wq"], f32)
    wk = np.asarray(inputs["wk"], f32)
    wv = np.asarray(inputs["wv"], f32)
    wo = np.asarray(inputs["wo"], f32)
    win = np.asarray(inputs["w_in"], f32)
    bin_ = np.asarray(inputs["b_in"], f32)
    wout = np.asarray(inputs["w_out"], f32)
    bout = np.asarray(inputs["b_out"], f32)
    sf = np.asarray(inputs["lnf_scale"], f32)
    bf_ = np.asarray(inputs["lnf_bias"], f32)
    wproj = np.asarray(inputs["w_proj"], f32)
    bproj = np.asarray(inputs["b_proj"], f32)

    ctx_t = np.ascontiguousarray(ctx.reshape(NTT, 128).T).astype(np.int32)

    pos = np.arange(T, dtype=f32)
    inv_freq = 1.0 / (10000.0 ** (np.arange(0, ROT, 2, dtype=f32) / ROT))
    freqs = pos[:, None] * inv_freq[None, :]          # [T, 32]
    sin = np.repeat(np.sin(freqs), 2, axis=-1).T      # [64, T]
    cos = np.repeat(np.cos(freqs), 2, axis=-1).T
    sinT = np.ascontiguousarray(np.tile(sin, (2, 1))).astype(BF)
    cosT = np.ascontiguousarray(np.tile(cos, (2, 1))).astype(BF)
    P = np.zeros((128, 128), f32)
    for i in range(64):
        P[2 * i, 2 * i + 1] = -1.0
        P[2 * i + 1, 2 * i] = 1.0
    pmat = np.ascontiguousarray(P.T).astype(BF)

    # causal mask tiles: offset o = 0,128,256,384; M[kp, q] = kp+o <= q
    kp = np.arange(128)[:, None]
    qq = np.arange(QCH)[None, :]
    masks = [(kp + o <= qq).astype(f32) for o in (0, 128, 256, 384)]
    maskc = np.ascontiguousarray(np.concatenate(masks, axis=1)).astype(BF)

    wproj_eff = sf[:, None] * wproj
    pb_full = bf_ @ wproj + bproj                      # [V]
    wproj_pad = np.zeros((D, VPAD), f32)
    wproj_pad[:, :V] = wproj_eff
    pb_pad = np.zeros(VPAD, f32)
    pb_pad[:V] = pb_full

    def center(w_eff):
        return w_eff - w_eff.mean(axis=0, keepdims=True)

    maps = []
    for c in range(NCORES):
        hsl = slice(c * HD, (c + 1) * HD)
        fsl = slice(c * FC, (c + 1) * FC)
        vsl = slice(c * VP, (c + 1) * VP)
        m = {
            "ctx_t": ctx_t,
            "embed": embed_w.astype(BF),
            "cosT": cosT,
            "sinT": sinT,
            "pmat": pmat,
            "maskc": maskc,
            "pb": pb_pad[vsl][None, :].astype(f32),
            "wproj": _kmajor(wproj_pad[:, vsl]).astype(BF),
        }
        wq_l, wk_l, wv_l, wo_l = [], [], [], []
        win_l, wout_l = [], []
        wors_l, wouts_l = [], []
        qb_l, kb_l, aob_l, hb_l, mob_l = [], [], [], [], []
        absum_l = []
        for l in range(L):
            wq_eff = center(s1[l][:, None] * wq[l])
            wk_eff = center(s1[l][:, None] * wk[l])
            wv_eff = center(s1[l][:, None] * wv[l])
            win_eff = center(s2[l][:, None] * win[l])
            wq_l.append(_kmajor(wq_eff[:, hsl]))
            wk_l.append(_kmajor(wk_eff[:, hsl]))
            wv_l.append(_kmajor(wv_eff[:, hsl]))
            wo_l.append(wo[l][hsl, :])
            win_l.append(_kmajor(win_eff[:, fsl]))
            wout_l.append(_kmajor(wout[l][fsl, :]))
            wors_l.append(wo[l][hsl, :].sum(axis=1)[:, None])
            wouts_l.append(
                wout[l][fsl, :].sum(axis=1).reshape(NFT, 128).T)
            qb_l.append((b1[l] @ wq[l])[hsl][:, None])
            kb_l.append((b1[l] @ wk[l])[hsl][:, None])
            aob_full = (b1[l] @ wv[l]) @ wo[l]
            aob_l.append(aob_full.reshape(NKT, 128).T / NCORES)
            hb_l.append(((b2[l] @ win[l]) + bin_[l])[fsl].reshape(NFT,
                                                                  128).T)
            mob_l.append(bout[l].reshape(NKT, 128).T / NCORES)
            absum_l.append(np.array([[aob_full.sum(), bout[l].sum()]],
                                    dtype=f32) / NCORES)
        m["wq"] = np.ascontiguousarray(np.stack(wq_l)).astype(BF)
        m["wk"] = np.ascontiguousarray(np.stack(wk_l)).astype(BF)
        m["wv"] = np.ascontiguousarray(np.stack(wv_l)).astype(BF)
        m["wo"] = np.ascontiguousarray(np.stack(wo_l)).astype(BF)
        m["win"] = np.ascontiguousarray(np.stack(win_l)).astype(BF)
        m["wout"] = np.ascontiguousarray(np.stack(wout_l)).astype(BF)
        m["wors"] = np.ascontiguousarray(np.stack(wors_l)).astype(BF)
        m["wouts"] = np.ascontiguousarray(np.stack(wouts_l)).astype(BF)
        m["qb"] = np.ascontiguousarray(np.stack(qb_l)).astype(f32)
        m["kb"] = np.ascontiguousarray(np.stack(kb_l)).astype(f32)
        m["aob"] = np.ascontiguousarray(np.stack(aob_l)).astype(f32)
        m["hb"] = np.ascontiguousarray(np.stack(hb_l)).astype(f32)
        m["mob"] = np.ascontiguousarray(np.stack(mob_l)).astype(f32)
        m["absum"] = np.ascontiguousarray(np.stack(absum_l)).astype(f32)
        maps.append(m)
    return maps


def _get_compiled():
    if "nc" not in _CACHE:
        _CACHE["nc"] = _build()
    return _CACHE["nc"]


def kernel(**inputs):
    from concourse.bass_utils import run_bass_kernel_spmd
    nc = _get_compiled()
    maps = _prep_inputs(inputs)
    res = run_bass_kernel_spmd(nc, maps, core_ids=list(range(NCORES)),
                               trace=False)
    logits = np.concatenate([res.results[c]["out"][0]
                             for c in range(NCORES)])[:V]
    return logits.reshape(1, 1, V).astype(np.float32)


def run_traced(inputs):
    """Like kernel() but with NTFF tracing; returns (logits, results)."""
    from concourse.bass_utils import run_bass_kernel_spmd
    nc = _get_compiled()
    maps = _prep_inputs(inputs)
    res = run_bass_kernel_spmd(nc, maps, core_ids=list(range(NCORES)),
                               trace=True)
    logits = np.concatenate([res.results[c]["out"][0]
                             for c in range(NCORES)])[:V]
    return logits.reshape(1, 1, V).astype(np.float32), res
